# revision 5
# baseline (speedup 1.0000x reference)
"""3-layer Elman RNN (tanh) Trainium2 kernel.

Model: x(512,2048) int -> emb(27,20) lookup -> RNN 20->32 -> 32->64 -> 64->64
       -> FC 64->26.  Output (512, 2048, 26) f32.

Strategy (per core, batch sharded 8 ways -> 256 batch/core, split into two
ping-pong halves of 128 so ACT and PE overlap across the serial recurrence):

All three layers advance in a skewed pipeline: at macro-step s, layer 1
processes t=s, layer 2 t=s-1, layer 3 t=s-2.  Per half-step one PSUM tile
P[128, 256] holds all three pre-activations:
  P[:, 0:128]    = pre2 (partitions 0..63) and pre3 (partitions 64..127)
  P[0:32,128:256]= pre1 (rest of that region is zeroed by a padded matmul)
filled by 4 small matmuls, then ONE ACT tanh op produces the next state tile
HNEW[128, 256] (same layout).  Layer-1's embedding+input-proj collapses to a
27->32 matmul against one-hot vectors (host-built, DMA'd in); its bias is
folded into the one-hot table (one-hot rows sum to 1), corrected for the
ACT bias vector which carries layer-2/3 biases.  FC runs in bulk per 2-step
chunk off the critical path; output written [26, T*B] per core and
reassembled on host.
"""

import os
import sys

sys.path.insert(0, "/opt/trn_rl_repo")

import numpy as np

import concourse.bacc as bacc
import concourse.tile as tile
from concourse import mybir

T = int(os.environ.get("RNN_T", "512"))  # env override only for debugging
B = 2048
NCORES = 8
BC = B // NCORES          # batch per core = 256
HB = BC // 2              # half-batch = 128
VOCAB, EMB, H1, H2, H3, OUT = 27, 20, 32, 64, 64, 26
S = T + 2                 # macro steps incl. pipeline flush

MM_DT = mybir.dt.bfloat16     # matmul operand dtype (states/weights)

import ml_dtypes  # noqa: E402

_NP_OF = {mybir.dt.bfloat16: ml_dtypes.bfloat16, mybir.dt.float32: np.float32}


def _build_nc():
    nc = bacc.Bacc()
    f32 = mybir.dt.float32
    mdt = MM_DT

    oh_d = nc.dram_tensor("oh", [VOCAB, T * BC], mdt, kind="ExternalInput")
    la_d = nc.dram_tensor("la", [H2 + H3, H2 + H3], mdt, kind="ExternalInput")
    lb_d = nc.dram_tensor("lb", [H1, H2], mdt, kind="ExternalInput")
    lc_d = nc.dram_tensor("lc", [H1, H1], mdt, kind="ExternalInput")
    le_d = nc.dram_tensor("le", [VOCAB, 128], mdt, kind="ExternalInput")
    lf_d = nc.dram_tensor("lf", [H3, OUT], mdt, kind="ExternalInput")
    b23_d = nc.dram_tensor("b23", [128, 1], f32, kind="ExternalInput")
    bfc_d = nc.dram_tensor("bfc", [OUT, 1], f32, kind="ExternalInput")
    o_d = nc.dram_tensor("o", [OUT, T * BC], f32, kind="ExternalOutput")

    with tile.TileContext(nc) as tc:
        with (
            tc.tile_pool(name="wpool", bufs=1) as wpool,
            tc.tile_pool(name="hpool", bufs=6) as hpool,
            tc.tile_pool(name="ohpool", bufs=3) as ohpool,
            tc.tile_pool(name="h3pool", bufs=2) as h3pool,
            tc.tile_pool(name="opool", bufs=3) as opool,
            tc.tile_pool(name="ppool", bufs=4, space="PSUM") as ppool,
            tc.tile_pool(name="fcpool", bufs=2, space="PSUM") as fcpool,
        ):
            la = wpool.tile([H2 + H3, H2 + H3], mdt)
            lb = wpool.tile([H1, H2], mdt)
            lc = wpool.tile([H1, H1], mdt)
            le = wpool.tile([VOCAB, 128], mdt)
            lf = wpool.tile([H3, OUT], mdt)
            b23 = wpool.tile([128, 1], f32)
            bfc = wpool.tile([OUT, 1], f32)
            for t_, d_ in ((la, la_d), (lb, lb_d), (lc, lc_d), (le, le_d),
                           (lf, lf_d), (b23, b23_d), (bfc, bfc_d)):
                nc.sync.dma_start(t_[:], d_[:])

            zst = wpool.tile([128, 2 * HB], mdt)   # zero initial state
            nc.vector.memset(zst[:], 0.0)
            zoh = wpool.tile([VOCAB, HB], mdt)     # zero one-hot for flush steps
            nc.vector.memset(zoh[:], 0.0)

            hprev = [zst, zst]
            oht = None
            h3buf = None
            tanh = mybir.ActivationFunctionType.Tanh

            for s in range(S):
                if s % 2 == 0 and s < T:
                    g = s // 2
                    oht = ohpool.tile([VOCAB, 4 * HB], mdt)
                    nc.sync.dma_start(oht[:], oh_d[:, 4 * HB * g:4 * HB * (g + 1)])
                for half in range(2):
                    hp = hprev[half]
                    p = ppool.tile([128, 2 * HB], f32)
                    # pre1 region [0:128, HB:2HB]: one-hot matmul zero-pads
                    # partitions 32..127, then the h1 recurrence accumulates.
                    if s < T:
                        o0 = (s % 2) * 2 * HB + half * HB
                        ohs = oht[:, o0:o0 + HB]
                    else:
                        ohs = zoh[:]
                    nc.tensor.matmul(p[:, HB:2 * HB], le[:], ohs, start=True, stop=False)
                    nc.tensor.matmul(p[0:H1, HB:2 * HB], lc[:], hp[0:H1, HB:2 * HB],
                                     start=False, stop=True)
                    # pre2/pre3 region [0:128, 0:HB]
                    nc.tensor.matmul(p[:, 0:HB], la[:], hp[:, 0:HB], start=True, stop=False)
                    nc.tensor.matmul(p[0:H2, 0:HB], lb[:], hp[0:H1, HB:2 * HB],
                                     start=False, stop=True)
                    hn = hpool.tile([128, 2 * HB], mdt)
                    nc.scalar.activation(hn[:], p[:], tanh, bias=b23[:])
                    if s == 0:
                        nc.vector.memset(hn[:, 0:HB], 0.0)       # H2, H3 invalid
                    elif s == 1:
                        nc.vector.memset(hn[H2:128, 0:HB], 0.0)  # H3 invalid
                    hprev[half] = hn
                    # collect h3 (valid output for t3 = s-2)
                    if s >= 2:
                        j = 2 * ((s - 2) % 2) + half
                        if j == 0:
                            h3buf = h3pool.tile([H3, 4 * HB], mdt)
                        nc.vector.tensor_copy(h3buf[:, HB * j:HB * (j + 1)],
                                              hn[H2:128, 0:HB])
                        if j == 3:
                            c = (s - 2) // 2
                            fco = fcpool.tile([OUT, 4 * HB], f32)
                            nc.tensor.matmul(fco[:], lf[:], h3buf[:], start=True, stop=True)
                            outs = opool.tile([OUT, 4 * HB], f32)
                            nc.vector.tensor_scalar_add(outs[:], fco[:], bfc[:])
                            nc.sync.dma_start(o_d[:, 4 * HB * c:4 * HB * (c + 1)], outs[:])
    nc.compile()
    return nc


_NC_CACHE = None


def _get_nc():
    global _NC_CACHE
    if _NC_CACHE is None:
        _NC_CACHE = _build_nc()
    return _NC_CACHE


def _prep_inputs(inputs):
    npdt = _NP_OF[MM_DT]
    f32 = np.float32
    x = np.asarray(inputs["x"]).astype(np.int64)            # (T, B)
    emb = np.asarray(inputs["emb"], f32)
    W_ih1 = np.asarray(inputs["W_ih1"], f32)
    W_hh1 = np.asarray(inputs["W_hh1"], f32)
    b1 = np.asarray(inputs["b_ih1"], f32) + np.asarray(inputs["b_hh1"], f32)
    W_ih2 = np.asarray(inputs["W_ih2"], f32)
    W_hh2 = np.asarray(inputs["W_hh2"], f32)
    b2 = np.asarray(inputs["b_ih2"], f32) + np.asarray(inputs["b_hh2"], f32)
    W_ih3 = np.asarray(inputs["W_ih3"], f32)
    W_hh3 = np.asarray(inputs["W_hh3"], f32)
    b3 = np.asarray(inputs["b_ih3"], f32) + np.asarray(inputs["b_hh3"], f32)
    W_fc = np.asarray(inputs["W_fc"], f32)
    b_fc = np.asarray(inputs["b_fc"], f32)

    # lhsT blocks (stationary operands, [K, M])
    la = np.zeros((H2 + H3, H2 + H3), f32)
    la[0:H2, 0:H2] = W_hh2.T
    la[0:H2, H2:] = W_ih3.T
    la[H2:, H2:] = W_hh3.T
    lb = W_ih2.T.copy()                                      # [32, 64]
    lc = W_hh1.T.copy()                                      # [32, 32]
    # EW'' table: emb @ W_ih1^T + b1, minus the b23[:32] the ACT bias adds
    ew = emb @ W_ih1.T + b1[None, :] - b2[None, 0:H1]        # [27, 32]
    le = np.zeros((VOCAB, 128), f32)
    le[:, 0:H1] = ew
    lf = W_fc.T.copy()                                       # [64, 26]
    b23 = np.concatenate([b2, b3]).reshape(128, 1).astype(f32)
    bfc = b_fc.reshape(OUT, 1).astype(f32)

    shared = {
        "la": la.astype(npdt), "lb": lb.astype(npdt), "lc": lc.astype(npdt),
        "le": le.astype(npdt), "lf": lf.astype(npdt),
        "b23": b23, "bfc": bfc,
    }
    in_maps = []
    for core in range(NCORES):
        xc = x[:, core * BC:(core + 1) * BC]                 # (T, BC)
        # one-hot [27, T*BC], free order (t, half, b)
        oh = (xc.reshape(T * BC)[None, :] == np.arange(VOCAB)[:, None])
        in_maps.append(dict(shared, oh=np.ascontiguousarray(oh.astype(npdt))))
    return in_maps


def _assemble(results):
    cores = []
    for core in range(NCORES):
        o = results[core]["o"]                               # [26, T*BC]
        cores.append(o.reshape(OUT, T, BC).transpose(1, 2, 0))
    return np.ascontiguousarray(np.concatenate(cores, axis=1), dtype=np.float32)


def _run(inputs, **spmd_kwargs):
    """Returns (output, BassKernelResults). spmd_kwargs e.g. trace=True."""
    from concourse.bass_utils import run_bass_kernel_spmd
    nc = _get_nc()
    in_maps = _prep_inputs(inputs)
    res = run_bass_kernel_spmd(nc, in_maps, core_ids=list(range(NCORES)),
                               **spmd_kwargs)
    return _assemble(res.results), res


def kernel(**inputs) -> np.ndarray:
    return _run(inputs)[0]


if __name__ == "__main__":
    import reference as R
    ins = {k: np.asarray(v) for k, v in R.setup_inputs().items()}
    got = kernel(**ins)
    import jax.numpy as jnp
    want = np.asarray(R.reference(**{k: jnp.asarray(v) for k, v in ins.items()}))
    err = np.abs(got - want)
    print("absmax", err.max(), "rel", err.max() / np.abs(want).max())


# revision 14
# speedup vs baseline: 1.0841x; 1.0841x over previous
"""3-layer Elman RNN (tanh) Trainium2 kernel.

Model: x(512,2048) int -> emb(27,20) lookup -> RNN 20->32 -> 32->64 -> 64->64
       -> FC 64->26.  Output (512, 2048, 26) f32.

Strategy (per core, batch sharded 8 ways -> 256 batch/core, split into two
ping-pong halves of 128 so ACT and PE overlap across the serial recurrence):

All three layers advance in a skewed pipeline: at macro-step s, layer 1
processes t=s, layer 2 t=s-1, layer 3 t=s-2.  Per half-step one PSUM tile
P[128, 256] holds all three pre-activations:
  P[:, 0:128]    = pre2 (partitions 0..63) and pre3 (partitions 64..127)
  P[0:32,128:256]= pre1 (rest of that region is zeroed by a padded matmul)
filled by 4 small matmuls, then ONE ACT tanh op produces the next state tile
HNEW[128, 256] (same layout).  Layer-1's embedding+input-proj collapses to a
27->32 matmul against one-hot vectors (host-built, DMA'd in); its bias is
folded into the one-hot table (one-hot rows sum to 1), corrected for the
ACT bias vector which carries layer-2/3 biases.  FC runs in bulk per 2-step
chunk off the critical path; output written [26, T*B] per core and
reassembled on host.
"""

import os
import sys

sys.path.insert(0, "/opt/trn_rl_repo")

import numpy as np

import concourse.bacc as bacc
import concourse.tile as tile
from concourse import mybir

T = int(os.environ.get("RNN_T", "512"))  # env override only for debugging
B = 2048
NCORES = 8
BC = B // NCORES          # batch per core = 256
HB = BC // 2              # half-batch = 128
VOCAB, EMB, H1, H2, H3, OUT = 27, 20, 32, 64, 64, 26
S = T + 2                 # macro steps incl. pipeline flush

MM_DT = mybir.dt.bfloat16     # matmul operand dtype (states/weights)

import ml_dtypes  # noqa: E402

# (walrus --enable-ldw-opt=true rejects bass-emitted InstLdweights; the
# per-matmul LDWEIGHTS reload is unavoidable at the ISA level here.)

_NP_OF = {mybir.dt.bfloat16: ml_dtypes.bfloat16, mybir.dt.float32: np.float32}


def _build_nc():
    nc = bacc.Bacc()
    f32 = mybir.dt.float32
    mdt = MM_DT

    oh_d = nc.dram_tensor("oh", [VOCAB, T * BC], mdt, kind="ExternalInput")
    la_d = nc.dram_tensor("la", [H2 + H3, H2 + H3], mdt, kind="ExternalInput")
    lb_d = nc.dram_tensor("lb", [H1, H2], mdt, kind="ExternalInput")
    lc_d = nc.dram_tensor("lc", [H1, H1], mdt, kind="ExternalInput")
    le_d = nc.dram_tensor("le", [VOCAB, H1], mdt, kind="ExternalInput")
    lf_d = nc.dram_tensor("lf", [H3, OUT], mdt, kind="ExternalInput")
    b23_d = nc.dram_tensor("b23", [128, 1], f32, kind="ExternalInput")
    bfc_d = nc.dram_tensor("bfc", [OUT, 1], f32, kind="ExternalInput")
    o_d = nc.dram_tensor("o", [OUT, T * BC], f32, kind="ExternalOutput")

    with tile.TileContext(nc) as tc:
        with (
            tc.tile_pool(name="wpool", bufs=1) as wpool,
            tc.tile_pool(name="hpool", bufs=6) as hpool,
            tc.tile_pool(name="ohpool", bufs=3) as ohpool,
            tc.tile_pool(name="h3pool", bufs=2) as h3pool,
            tc.tile_pool(name="opool", bufs=3) as opool,
            tc.tile_pool(name="ppool", bufs=4, space="PSUM") as ppool,
            tc.tile_pool(name="fcpool", bufs=2, space="PSUM") as fcpool,
            tc.tile_pool(name="warmp", bufs=1, space="PSUM") as warmp,
        ):
            la = wpool.tile([H2 + H3, H2 + H3], mdt)
            lb = wpool.tile([H1, H2], mdt)
            lc = wpool.tile([H1, H1], mdt)
            le = wpool.tile([VOCAB, H1], mdt)
            lf = wpool.tile([H3, OUT], mdt)
            b23 = wpool.tile([128, 1], f32)
            bfc = wpool.tile([OUT, 1], f32)
            for t_, d_ in ((la, la_d), (lb, lb_d), (lc, lc_d), (le, le_d),
                           (lf, lf_d), (b23, b23_d), (bfc, bfc_d)):
                nc.sync.dma_start(t_[:], d_[:])

            zst = wpool.tile([128, 2 * HB], mdt)   # zero initial state
            nc.vector.memset(zst[:], 0.0)
            zoh = wpool.tile([VOCAB, HB], mdt)     # zero one-hot for flush steps
            nc.vector.memset(zoh[:], 0.0)

            # PE warmup: ~5us of back-to-back matmuls trips the HAM clock
            # gate to 8/8 (2.4 GHz) before the latency-critical loop begins.
            warm = wpool.tile([128, 512], mdt)
            nc.vector.memset(warm[:], 0.0)
            wp = warmp.tile([128, 512], mybir.dt.float32)
            for _ in range(12):
                nc.tensor.matmul(wp[:], warm[:, 0:128], warm[:], start=True, stop=True)

            # Prime all ppool PSUM slots: the [H1:128, HB:2HB] region of each
            # is never written by the matmuls but is read by the packed ACT;
            # zero it once so tanh sees finite values (NaN/garbage there can
            # wedge the runtime's numerical notifications).
            for _ in range(4):
                pp = ppool.tile([128, 2 * HB], mybir.dt.float32, tag="p")
                nc.vector.memset(pp[:], 0.0)

            hprev = [zst, zst]
            oht = None
            h3buf = None
            tanh = mybir.ActivationFunctionType.Tanh

            for s in range(S):
                if s % 2 == 0 and s < T:
                    g = s // 2
                    oht = ohpool.tile([VOCAB, 4 * HB], mdt)
                    nc.sync.dma_start(oht[:], oh_d[:, 4 * HB * g:4 * HB * (g + 1)])
                for half in range(2):
                    hp = hprev[half]
                    p = ppool.tile([128, 2 * HB], f32)
                    # pre1 region [0:128, HB:2HB]: one-hot matmul zero-pads
                    # partitions 32..127, then the h1 recurrence accumulates.
                    if s < T:
                        o0 = (s % 2) * 2 * HB + half * HB
                        ohs = oht[:, o0:o0 + HB]
                    else:
                        ohs = zoh[:]
                    nc.tensor.matmul(p[0:H1, HB:2 * HB], le[:], ohs, start=True, stop=False)
                    nc.tensor.matmul(p[0:H1, HB:2 * HB], lc[:], hp[0:H1, HB:2 * HB],
                                     start=False, stop=True)
                    # pre2/pre3 region [0:128, 0:HB]
                    nc.tensor.matmul(p[:, 0:HB], la[:], hp[:, 0:HB], start=True, stop=False)
                    nc.tensor.matmul(p[0:H2, 0:HB], lb[:], hp[0:H1, HB:2 * HB],
                                     start=False, stop=True)
                    hn = hpool.tile([128, 2 * HB], mdt)
                    nc.scalar.activation(hn[:], p[:], tanh, bias=b23[:])
                    if s == 0:
                        nc.vector.memset(hn[:, 0:HB], 0.0)       # H2, H3 invalid
                    elif s == 1:
                        nc.vector.memset(hn[H2:128, 0:HB], 0.0)  # H3 invalid
                    hprev[half] = hn
                    # collect h3 (valid output for t3 = s-2)
                    if s >= 2:
                        j = 2 * ((s - 2) % 2) + half
                        if j == 0:
                            h3buf = h3pool.tile([H3, 4 * HB], mdt)
                        nc.vector.tensor_copy(h3buf[:, HB * j:HB * (j + 1)],
                                              hn[H2:128, 0:HB])
                        if j == 3:
                            c = (s - 2) // 2
                            fco = fcpool.tile([OUT, 4 * HB], f32)
                            nc.tensor.matmul(fco[:], lf[:], h3buf[:], start=True, stop=True)
                            outs = opool.tile([OUT, 4 * HB], f32)
                            nc.vector.tensor_scalar_add(outs[:], fco[:], bfc[:])
                            nc.sync.dma_start(o_d[:, 4 * HB * c:4 * HB * (c + 1)], outs[:])
    nc.compile()
    return nc


_NC_CACHE = None


def _get_nc():
    global _NC_CACHE
    if _NC_CACHE is None:
        _NC_CACHE = _build_nc()
    return _NC_CACHE


def _prep_inputs(inputs):
    npdt = _NP_OF[MM_DT]
    f32 = np.float32
    x = np.asarray(inputs["x"]).astype(np.int64)            # (T, B)
    emb = np.asarray(inputs["emb"], f32)
    W_ih1 = np.asarray(inputs["W_ih1"], f32)
    W_hh1 = np.asarray(inputs["W_hh1"], f32)
    b1 = np.asarray(inputs["b_ih1"], f32) + np.asarray(inputs["b_hh1"], f32)
    W_ih2 = np.asarray(inputs["W_ih2"], f32)
    W_hh2 = np.asarray(inputs["W_hh2"], f32)
    b2 = np.asarray(inputs["b_ih2"], f32) + np.asarray(inputs["b_hh2"], f32)
    W_ih3 = np.asarray(inputs["W_ih3"], f32)
    W_hh3 = np.asarray(inputs["W_hh3"], f32)
    b3 = np.asarray(inputs["b_ih3"], f32) + np.asarray(inputs["b_hh3"], f32)
    W_fc = np.asarray(inputs["W_fc"], f32)
    b_fc = np.asarray(inputs["b_fc"], f32)

    # lhsT blocks (stationary operands, [K, M])
    la = np.zeros((H2 + H3, H2 + H3), f32)
    la[0:H2, 0:H2] = W_hh2.T
    la[0:H2, H2:] = W_ih3.T
    la[H2:, H2:] = W_hh3.T
    lb = W_ih2.T.copy()                                      # [32, 64]
    lc = W_hh1.T.copy()                                      # [32, 32]
    # EW'' table: emb @ W_ih1^T + b1, minus the b23[:32] the ACT bias adds
    le = emb @ W_ih1.T + b1[None, :] - b2[None, 0:H1]        # [27, 32]
    lf = W_fc.T.copy()                                       # [64, 26]
    b23 = np.concatenate([b2, b3]).reshape(128, 1).astype(f32)
    bfc = b_fc.reshape(OUT, 1).astype(f32)

    shared = {
        "la": la.astype(npdt), "lb": lb.astype(npdt), "lc": lc.astype(npdt),
        "le": le.astype(npdt), "lf": lf.astype(npdt),
        "b23": b23, "bfc": bfc,
    }
    in_maps = []
    for core in range(NCORES):
        xc = x[:, core * BC:(core + 1) * BC]                 # (T, BC)
        # one-hot [27, T*BC], free order (t, half, b)
        oh = (xc.reshape(T * BC)[None, :] == np.arange(VOCAB)[:, None])
        in_maps.append(dict(shared, oh=np.ascontiguousarray(oh.astype(npdt))))
    return in_maps


def _assemble(results):
    cores = []
    for core in range(NCORES):
        o = results[core]["o"]                               # [26, T*BC]
        cores.append(o.reshape(OUT, T, BC).transpose(1, 2, 0))
    return np.ascontiguousarray(np.concatenate(cores, axis=1), dtype=np.float32)


def _run(inputs, **spmd_kwargs):
    """Returns (output, BassKernelResults). spmd_kwargs e.g. trace=True."""
    from concourse.bass_utils import run_bass_kernel_spmd
    nc = _get_nc()
    in_maps = _prep_inputs(inputs)
    res = run_bass_kernel_spmd(nc, in_maps, core_ids=list(range(NCORES)),
                               **spmd_kwargs)
    return _assemble(res.results), res


def kernel(**inputs) -> np.ndarray:
    return _run(inputs)[0]


if __name__ == "__main__":
    import reference as R
    ins = {k: np.asarray(v) for k, v in R.setup_inputs().items()}
    got = kernel(**ins)
    import jax.numpy as jnp
    want = np.asarray(R.reference(**{k: jnp.asarray(v) for k, v in ins.items()}))
    err = np.abs(got - want)
    print("absmax", err.max(), "rel", err.max() / np.abs(want).max())
